# revision 15
# baseline (speedup 1.0000x reference)
"""DeepseekV3 MoE layer on 8 Trainium2 NeuronCores (Bass/Tile).

Strategy (expert-parallel, host-routed):
  - Router (h @ gate_w.T, sigmoid, top-8-of-16, weight norm) runs on host in
    fp32 — it is ~134 MFLOP, negligible, and data-dependent control flow is
    hostile to the static device ISA.
  - The 16 routed experts are sharded 2-per-core.  The host gathers each
    expert's tokens (capacity-padded to a multiple of 128), ships them
    transposed ([H, C] "feature-major") in bf16, and the device runs the
    fused SwiGLU chain  Y = (silu(X@Wg) * (X@Wu)) @ Wd  with fp32 PSUM
    accumulation, scaling rows by the combine weight on the way out.
    Experts are assigned to the two per-core slots by descending token
    count, so slot 0 compiles with capacity C0 >= C1 of slot 1 and the
    padding waste stays small.
  - The shared expert is tensor-parallel over its intermediate dim: each
    core owns an IS/8 = 256 slice of sWg/sWu/sWd (2.5 MB instead of a
    25 MB replica — the replicated variant made the kernel prologue
    HBM-bound) and produces a full [T, H] partial that the host sums.
  - Host scatters per-expert outputs back (indices are unique per expert)
    and adds the shared partials.  All combination arithmetic is fp32.

Device notes:
  - Activations stay feature-major [feat, tokens] so gate/up projections
    need no transposes and the down projection consumes A^T directly as the
    stationary operand, returning token-major output tiles.
  - PSUM is managed as 8 single-bank [128, 512] fp32 tiles shared by every
    accumulation group.
  - DMA issue is split across the two HWDGE engines (sync + scalar) —
    each engine owns one hardware queue, so this doubles queue parallelism
    and keeps the shared-expert stream off the weight-fetch path.
"""

import math

import numpy as np
import ml_dtypes

import concourse.mybir as mybir
import concourse.tile as tile
from concourse import bacc
from concourse.bass_utils import run_bass_kernel_spmd

BF16 = ml_dtypes.bfloat16
F32 = mybir.dt.float32
BF = mybir.dt.bfloat16
ACT = mybir.ActivationFunctionType

H = 2048          # hidden size
IM = 1024         # routed expert intermediate
E = 16            # routed experts
TOPK = 8
T = 2048          # tokens (B=1, L=2048)
SCALE = 2.5
NCORES = 8
EL = E // NCORES  # experts per core
IS = 2048         # shared expert intermediate (IM * n_shared)
TSH = T // NCORES # shared-expert tokens per core
P = 128


def _chunks(total, size):
    return [(o, min(size, total - o)) for o in range(0, total, size)]


def build_program(caps, h=H, im=IM, iss=IS // NCORES, tt=T,
                  n_devices=NCORES):
    """Build + bass-compile the per-core SPMD program.

    caps: per-expert-slot token capacities (multiples of 128), len == EL.
    iss:  this core's slice of the shared-expert intermediate dim.
    tt:   token count for the shared expert (full sequence).
    """
    el = len(caps)
    assert all(c % P == 0 for c in caps)
    assert h % 512 == 0 and im % P == 0 and iss % P == 0
    KH = h // P            # contraction tiles over hidden dim
    MI = im // P           # routed intermediate partition tiles (<= 8)
    assert MI <= 8
    HN = h // 512
    ISM = iss // P         # shared-slice intermediate partition tiles
    assert tt % 1024 == 0
    TH2 = tt // 2          # fused shared G/U pass processes token halves
    assert ISM * 2 * (TH2 // 512) <= 8
    CSUM = sum(caps)
    coff = [sum(caps[:i]) for i in range(el)]          # xt column offsets
    CT = [c // P for c in caps]
    ctoff = [sum(CT[:i]) for i in range(el)]           # wtk column offsets

    nc = bacc.Bacc("TRN2", target_bir_lowering=False, debug=False,
                   num_devices=n_devices)

    xt = nc.dram_tensor("xt", [h, CSUM], BF, kind="ExternalInput").ap()
    wtk = nc.dram_tensor("wtk", [P, sum(CT)], F32, kind="ExternalInput").ap()
    ht = nc.dram_tensor("ht", [h, tt], BF, kind="ExternalInput").ap()
    wg = nc.dram_tensor("wg", [el, h, im], BF, kind="ExternalInput").ap()
    wu = nc.dram_tensor("wu", [el, h, im], BF, kind="ExternalInput").ap()
    wd = nc.dram_tensor("wd", [el, im, h], BF, kind="ExternalInput").ap()
    swg = nc.dram_tensor("swg", [h, iss], BF, kind="ExternalInput").ap()
    swu = nc.dram_tensor("swu", [h, iss], BF, kind="ExternalInput").ap()
    swd = nc.dram_tensor("swd", [iss, h], BF, kind="ExternalInput").ap()
    yex = [nc.dram_tensor(f"yex{e}", [caps[e], h], F32,
                          kind="ExternalOutput").ap() for e in range(el)]
    ysh = nc.dram_tensor("ysh", [tt, h], F32, kind="ExternalOutput").ap()

    QK = 4                 # hidden-dim k-slabs fetched per DMA
    assert KH % QK == 0
    with tile.TileContext(nc) as tc:
        with (
            tc.tile_pool(name="consts", bufs=1) as consts,
            tc.tile_pool(name="wpool", bufs=2) as wpool,
            tc.tile_pool(name="xpool", bufs=4) as xpool,
            tc.tile_pool(name="gpool", bufs=1) as gpool,
            tc.tile_pool(name="apool", bufs=1) as apool,
            tc.tile_pool(name="htp", bufs=4) as htp,
            tc.tile_pool(name="stage", bufs=4) as stage,
            tc.tile_pool(name="psum", bufs=8, space="PSUM") as psum,
        ):
            # PE warm-up: the HAM clock gate releases only after ~3.4us of
            # sustained matmul activity; burn the initial DMA-wait time on a
            # dummy accumulation so real work starts at 2.4 GHz.
            warm = consts.tile([P, 256], BF, name="warm")
            nc.vector.memset(warm[:], 0.0)
            ps_w = psum.tile([P, 512], F32, name="ps", tag="ps")
            for i in range(40):
                nc.tensor.matmul(ps_w[:, :256], warm[:, :P], warm[:],
                                 start=(i == 0), stop=(i == 39))
            nc.vector.tensor_copy(warm[:, :P], ps_w[:, :P])

            # ------------- shared expert (tensor-parallel IS slice) -------
            # small resident weight slices; gate+up fused over one ht stream
            sgw = consts.tile([P, KH, iss], BF, name="sgw")
            suw = consts.tile([P, KH, iss], BF, name="suw")
            for q in range(KH // QK):
                nc.scalar.dma_start(
                    sgw[:, q * QK:(q + 1) * QK, :],
                    swg[q * QK * P:(q + 1) * QK * P, :]
                    .rearrange("(k p) i -> p k i", p=P))
                nc.scalar.dma_start(
                    suw[:, q * QK:(q + 1) * QK, :],
                    swu[q * QK * P:(q + 1) * QK * P, :]
                    .rearrange("(k p) i -> p k i", p=P))
            sdw = consts.tile([P, ISM, h], BF, name="sdw")
            nc.scalar.dma_start(sdw[:],
                                swd.rearrange("(k p) i -> p k i", p=P))

            wtk_sb = consts.tile([P, sum(CT)], F32, name="wtk_sb")
            nc.scalar.dma_start(wtk_sb[:], wtk[:, :])

            gss = consts.tile([P, ISM, tt], BF, name="gss")
            ass = consts.tile([P, ISM, tt], BF, name="ass")

            TN = TH2 // 512        # 512-token chunks per half
            for th in range(2):
                t0 = th * TH2
                psg = [[psum.tile([P, 512], F32, name="ps", tag="ps")
                        for _ in range(TN)] for _ in range(ISM)]
                psu = [[psum.tile([P, 512], F32, name="ps", tag="ps")
                        for _ in range(TN)] for _ in range(ISM)]
                for k0 in range(KH // QK):
                    hsl = htp.tile([P, QK, TH2], BF, name="hsl",
                                   tag="hsl")
                    nc.sync.dma_start(
                        hsl, ht[k0 * QK * P:(k0 + 1) * QK * P,
                                t0:t0 + TH2]
                        .rearrange("(q p) t -> p q t", p=P))
                    for kq in range(QK):
                        k = k0 * QK + kq
                        for m in range(ISM):
                            for tn in range(TN):
                                nc.tensor.matmul(
                                    psg[m][tn],
                                    sgw[:, k, m * P:(m + 1) * P],
                                    hsl[:, kq, tn * 512:(tn + 1) * 512],
                                    start=(k == 0), stop=(k == KH - 1))
                                nc.tensor.matmul(
                                    psu[m][tn],
                                    suw[:, k, m * P:(m + 1) * P],
                                    hsl[:, kq, tn * 512:(tn + 1) * 512],
                                    start=(k == 0), stop=(k == KH - 1))
                for m in range(ISM):
                    for tn in range(TN):
                        sl_ = slice(t0 + tn * 512, t0 + (tn + 1) * 512)
                        nc.scalar.activation(gss[:, m, sl_], psg[m][tn][:],
                                             ACT.Silu)
                        nc.vector.tensor_mul(ass[:, m, sl_], gss[:, m, sl_],
                                             psu[m][tn][:])

            # shared down-projection: token-major [tt, h] partial
            for cm in range(tt // P):
                for hn in range(HN):
                    pst = psum.tile([P, 512], F32, name="ps", tag="ps")
                    for kk in range(ISM):
                        nc.tensor.matmul(
                            pst, ass[:, kk, cm * P:(cm + 1) * P],
                            sdw[:, kk, hn * 512:(hn + 1) * 512],
                            start=(kk == 0), stop=(kk == ISM - 1))
                    st = stage.tile([P, 512], F32, name="st", tag="st")
                    nc.scalar.activation(st[:], pst[:], ACT.Copy)
                    nc.scalar.dma_start(
                        ysh[cm * P:(cm + 1) * P, hn * 512:(hn + 1) * 512],
                        st[:])

            # ---------------- routed experts (expert-parallel) ------------
            for e in range(el):
                C = caps[e]
                nch = _chunks(C, 512)
                # gate projection
                wgt = wpool.tile([P, KH, im], BF, name="wmat", tag="wmat")
                nc.sync.dma_start(wgt[:],
                                  wg[e].rearrange("(k p) i -> p k i", p=P))
                gst = gpool.tile([P, MI, caps[0]], BF, name="gst")
                for (noff, nsz) in nch:
                    pse = [psum.tile([P, 512], F32, name="ps", tag="ps")
                           for _ in range(MI)]
                    for k0 in range(KH // QK):
                        xsl = xpool.tile([P, QK, 512], BF, name="xsl",
                                         tag="xsl")[:, :, :nsz]
                        nc.sync.dma_start(
                            xsl, xt[k0 * QK * P:(k0 + 1) * QK * P,
                                    coff[e] + noff:coff[e] + noff + nsz]
                            .rearrange("(q p) n -> p q n", p=P))
                        for kq in range(QK):
                            k = k0 * QK + kq
                            for m in range(MI):
                                nc.tensor.matmul(
                                    pse[m][:, :nsz],
                                    wgt[:, k, m * P:(m + 1) * P],
                                    xsl[:, kq, :],
                                    start=(k == 0), stop=(k == KH - 1))
                    for m in range(MI):
                        nc.scalar.activation(gst[:, m, noff:noff + nsz],
                                             pse[m][:, :nsz], ACT.Silu)

                # up projection (evicts through silu(G) * U)
                wut = wpool.tile([P, KH, im], BF, name="wmat", tag="wmat")
                nc.sync.dma_start(wut[:],
                                  wu[e].rearrange("(k p) i -> p k i", p=P))
                ast = apool.tile([P, MI, caps[0]], BF, name="ast")
                for (noff, nsz) in nch:
                    pse = [psum.tile([P, 512], F32, name="ps", tag="ps")
                           for _ in range(MI)]
                    for k0 in range(KH // QK):
                        xsl = xpool.tile([P, QK, 512], BF, name="xsl",
                                         tag="xsl")[:, :, :nsz]
                        nc.scalar.dma_start(
                            xsl, xt[k0 * QK * P:(k0 + 1) * QK * P,
                                    coff[e] + noff:coff[e] + noff + nsz]
                            .rearrange("(q p) n -> p q n", p=P))
                        for kq in range(QK):
                            k = k0 * QK + kq
                            for m in range(MI):
                                nc.tensor.matmul(
                                    pse[m][:, :nsz],
                                    wut[:, k, m * P:(m + 1) * P],
                                    xsl[:, kq, :],
                                    start=(k == 0), stop=(k == KH - 1))
                    for m in range(MI):
                        nc.vector.tensor_mul(ast[:, m, noff:noff + nsz],
                                             gst[:, m, noff:noff + nsz],
                                             pse[m][:, :nsz])

                # down projection, token-major out, combine-weight scaling
                wdt = wpool.tile([P, MI, h], BF, name="wmat", tag="wmat")
                nc.sync.dma_start(wdt[:],
                                  wd[e].rearrange("(k p) i -> p k i", p=P))
                for cm in range(C // P):
                    for hn in range(HN):
                        pst = psum.tile([P, 512], F32, name="ps", tag="ps")
                        for kk in range(MI):
                            nc.tensor.matmul(
                                pst, ast[:, kk, cm * P:(cm + 1) * P],
                                wdt[:, kk, hn * 512:(hn + 1) * 512],
                                start=(kk == 0), stop=(kk == MI - 1))
                        st = stage.tile([P, 512], F32, name="st", tag="st")
                        nc.scalar.activation(
                            st[:], pst[:], ACT.Copy,
                            scale=wtk_sb[:, ctoff[e] + cm:ctoff[e] + cm + 1])
                        nc.sync.dma_start(
                            yex[e][cm * P:(cm + 1) * P,
                                   hn * 512:(hn + 1) * 512], st[:])

    nc.compile()
    return nc


_prog_cache = {}

# Debug/timing hooks for the local test harness (harmless defaults for
# grading: no tracing, results kept only when asked for).
TRACE = False
TRACE_KWARGS = {}
LAST_RESULT = None


def _get_program(caps):
    if caps not in _prog_cache:
        _prog_cache[caps] = build_program(caps)
    return _prog_cache[caps]


def _route(h32, gate_w):
    """Host router: returns per-expert (token_idx, combine_weight)."""
    logits = h32 @ np.asarray(gate_w, np.float32).T            # [T, E]
    rw = 1.0 / (1.0 + np.exp(-logits))
    topk_idx = np.argsort(-rw, axis=-1, kind="stable")[:, :TOPK]
    topk_w = np.take_along_axis(rw, topk_idx, -1)
    topk_w = topk_w / (topk_w.sum(-1, keepdims=True) + 1e-6) * SCALE
    sel, wsel = [], []
    for e in range(E):
        mask = topk_idx == e                                    # [T, K]
        tok = np.nonzero(mask.any(-1))[0]
        sel.append(tok)
        wsel.append((topk_w * mask).sum(-1)[tok].astype(np.float32))
    return sel, wsel


def kernel(hidden_states, gate_w, Wg, Wu, Wd, sWg, sWu, sWd):
    h32 = np.asarray(hidden_states, np.float32).reshape(T, H)
    sel, wsel = _route(h32, gate_w)

    # Assign experts to (core, slot): slot 0 gets the 8 busiest experts so
    # slot capacities (compile-time constants) hug the actual counts.
    order = sorted(range(E), key=lambda e: -len(sel[e]))
    slot_experts = [order[:NCORES], order[NCORES:]]             # [slot][core]
    caps = tuple(
        max(P, int(math.ceil(max(len(sel[e]) for e in slot_experts[s]) / P))
            * P)
        for s in range(EL))
    nc = _get_program(caps)
    coff = [sum(caps[:i]) for i in range(EL)]
    CT = [c // P for c in caps]
    ctoff = [sum(CT[:i]) for i in range(EL)]

    ht_bf = np.ascontiguousarray(h32.T).astype(BF16)            # [H, T]
    swg_bf = np.asarray(sWg, np.float32).astype(BF16)
    swu_bf = np.asarray(sWu, np.float32).astype(BF16)
    swd_bf = np.asarray(sWd, np.float32).astype(BF16)
    wg_bf = np.asarray(Wg, np.float32).astype(BF16)
    wu_bf = np.asarray(Wu, np.float32).astype(BF16)
    wd_bf = np.asarray(Wd, np.float32).astype(BF16)
    ISS = IS // NCORES

    in_maps = []
    for c in range(NCORES):
        experts = [slot_experts[s][c] for s in range(EL)]
        xt = np.zeros((H, sum(caps)), BF16)
        wtk = np.zeros((P, sum(CT)), np.float32)
        for s, e in enumerate(experts):
            tok, w = sel[e], wsel[e]
            n = len(tok)
            xt[:, coff[s]:coff[s] + n] = ht_bf[:, tok]
            wcol = np.zeros(caps[s], np.float32)
            wcol[:n] = w
            wtk[:, ctoff[s]:ctoff[s] + CT[s]] = wcol.reshape(CT[s], P).T
        in_maps.append({
            "xt": xt,
            "wtk": wtk,
            "ht": ht_bf,
            "wg": np.ascontiguousarray(wg_bf[experts]),
            "wu": np.ascontiguousarray(wu_bf[experts]),
            "wd": np.ascontiguousarray(wd_bf[experts]),
            "swg": np.ascontiguousarray(swg_bf[:, c * ISS:(c + 1) * ISS]),
            "swu": np.ascontiguousarray(swu_bf[:, c * ISS:(c + 1) * ISS]),
            "swd": np.ascontiguousarray(swd_bf[c * ISS:(c + 1) * ISS, :]),
        })

    res = run_bass_kernel_spmd(nc, in_maps, list(range(NCORES)),
                               trace=TRACE, **TRACE_KWARGS)
    if TRACE:
        global LAST_RESULT
        LAST_RESULT = res

    out = res.results[0]["ysh"].astype(np.float32)
    for c in range(1, NCORES):
        out += res.results[c]["ysh"]
    for c in range(NCORES):
        for s in range(EL):
            e = slot_experts[s][c]
            tok = sel[e]
            out[tok] += res.results[c][f"yex{s}"][:len(tok)]

    return out.reshape(np.asarray(hidden_states).shape).astype(np.float32)


# revision 19
# speedup vs baseline: 1.0547x; 1.0547x over previous
"""DeepseekV3 MoE layer on 8 Trainium2 NeuronCores (Bass/Tile).

Strategy (expert-parallel, host-routed):
  - Router (h @ gate_w.T, sigmoid, top-8-of-16, weight norm) runs on host in
    fp32 — it is ~134 MFLOP, negligible, and data-dependent control flow is
    hostile to the static device ISA.
  - The 16 routed experts are sharded 2-per-core.  The host gathers each
    expert's tokens (capacity-padded to a multiple of 128), ships them
    transposed ([H, C] "feature-major") in bf16, and the device runs the
    fused SwiGLU chain  Y = (silu(X@Wg) * (X@Wu)) @ Wd  with fp32 PSUM
    accumulation, scaling rows by the combine weight on the way out.
    Experts are assigned to the two per-core slots by descending token
    count, so slot 0 compiles with capacity C0 >= C1 of slot 1 and the
    padding waste stays small.
  - The shared expert is sharded 4-way tensor-parallel over its
    intermediate dim x 2-way over tokens: each core owns an IS/4 = 512
    slice of sWg/sWu/sWd (5 MB instead of a 25 MB replica, which made the
    kernel prologue HBM-bound) for half the tokens, producing a [T/2, H]
    partial; the host sums 4 partials per token half.  The 512-deep
    down-projection keeps PSUM accumulation groups long enough to amortize
    eviction.
  - Host scatters per-expert outputs back (indices are unique per expert)
    and adds the shared partials.  All combination arithmetic is fp32.

Device notes:
  - Activations stay feature-major [feat, tokens] so gate/up projections
    need no transposes and the down projection consumes A^T directly as the
    stationary operand, returning token-major output tiles.
  - PSUM is managed as 8 single-bank [128, 512] fp32 tiles shared by every
    accumulation group.
  - DMA issue is split across the two HWDGE engines (sync + scalar) —
    each engine owns one hardware queue, so this doubles queue parallelism
    and keeps the shared-expert stream off the weight-fetch path.
"""

import math

import numpy as np
import ml_dtypes

import concourse.mybir as mybir
import concourse.tile as tile
from concourse import bacc
from concourse.bass_utils import run_bass_kernel_spmd

BF16 = ml_dtypes.bfloat16
F32 = mybir.dt.float32
BF = mybir.dt.bfloat16
ACT = mybir.ActivationFunctionType

H = 2048          # hidden size
IM = 1024         # routed expert intermediate
E = 16            # routed experts
TOPK = 8
T = 2048          # tokens (B=1, L=2048)
SCALE = 2.5
NCORES = 8
EL = E // NCORES  # experts per core
IS = 2048         # shared expert intermediate (IM * n_shared)
TSH = T // NCORES # shared-expert tokens per core
P = 128


def _chunks(total, size):
    return [(o, min(size, total - o)) for o in range(0, total, size)]


def build_program(caps, h=H, im=IM, iss=IS // 4, tt=T // 2,
                  n_devices=NCORES):
    """Build + bass-compile the per-core SPMD program.

    caps: per-expert-slot token capacities (multiples of 128), len == EL.
    iss:  this core's slice of the shared-expert intermediate dim.
    tt:   token count for the shared expert (full sequence).
    """
    el = len(caps)
    assert all(c % P == 0 for c in caps)
    assert h % 512 == 0 and im % P == 0 and iss % P == 0
    KH = h // P            # contraction tiles over hidden dim
    MI = im // P           # routed intermediate partition tiles (<= 8)
    assert MI <= 8
    HN = h // 512
    ISM = iss // P         # shared-slice intermediate partition tiles
    assert tt % 1024 == 0
    TH2 = tt // 2          # fused shared G/U pass processes token halves
    assert ISM * 2 * (TH2 // 512) <= 8
    CSUM = sum(caps)
    coff = [sum(caps[:i]) for i in range(el)]          # xt column offsets
    CT = [c // P for c in caps]
    ctoff = [sum(CT[:i]) for i in range(el)]           # wtk column offsets

    nc = bacc.Bacc("TRN2", target_bir_lowering=False, debug=False,
                   num_devices=n_devices)

    xt = nc.dram_tensor("xt", [h, CSUM], BF, kind="ExternalInput").ap()
    wtk = nc.dram_tensor("wtk", [P, sum(CT)], F32, kind="ExternalInput").ap()
    ht = nc.dram_tensor("ht", [h, tt], BF, kind="ExternalInput").ap()
    wg = nc.dram_tensor("wg", [el, h, im], BF, kind="ExternalInput").ap()
    wu = nc.dram_tensor("wu", [el, h, im], BF, kind="ExternalInput").ap()
    wd = nc.dram_tensor("wd", [el, im, h], BF, kind="ExternalInput").ap()
    swg = nc.dram_tensor("swg", [h, iss], BF, kind="ExternalInput").ap()
    swu = nc.dram_tensor("swu", [h, iss], BF, kind="ExternalInput").ap()
    swd = nc.dram_tensor("swd", [iss, h], BF, kind="ExternalInput").ap()
    yex = [nc.dram_tensor(f"yex{e}", [caps[e], h], F32,
                          kind="ExternalOutput").ap() for e in range(el)]
    ysh = nc.dram_tensor("ysh", [tt, h], F32, kind="ExternalOutput").ap()

    QK = 4                 # hidden-dim k-slabs fetched per DMA
    assert KH % QK == 0
    with tile.TileContext(nc) as tc:
        with (
            tc.tile_pool(name="consts", bufs=1) as consts,
            tc.tile_pool(name="wpool", bufs=2) as wpool,
            tc.tile_pool(name="xpool", bufs=4) as xpool,
            tc.tile_pool(name="gpool", bufs=1) as gpool,
            tc.tile_pool(name="apool", bufs=1) as apool,
            tc.tile_pool(name="htp", bufs=4) as htp,
            tc.tile_pool(name="stage", bufs=4) as stage,
            tc.tile_pool(name="psum", bufs=8, space="PSUM") as psum,
        ):
            # PE warm-up: the HAM clock gate releases only after ~3.4us of
            # sustained matmul activity; burn the initial DMA-wait time on a
            # dummy accumulation so real work starts at 2.4 GHz.
            warm = consts.tile([P, 256], BF, name="warm")
            nc.vector.memset(warm[:], 0.0)
            ps_w = psum.tile([P, 512], F32, name="ps", tag="ps")
            for i in range(40):
                nc.tensor.matmul(ps_w[:, :256], warm[:, :P], warm[:],
                                 start=(i == 0), stop=(i == 39))
            nc.vector.tensor_copy(warm[:, :P], ps_w[:, :P])

            # ------------- shared expert (tensor-parallel IS slice) -------
            # small resident weight slices; gate+up fused over one ht stream
            sgw = consts.tile([P, KH, iss], BF, name="sgw")
            suw = consts.tile([P, KH, iss], BF, name="suw")
            for q in range(KH // QK):
                nc.scalar.dma_start(
                    sgw[:, q * QK:(q + 1) * QK, :],
                    swg[q * QK * P:(q + 1) * QK * P, :]
                    .rearrange("(k p) i -> p k i", p=P))
                nc.scalar.dma_start(
                    suw[:, q * QK:(q + 1) * QK, :],
                    swu[q * QK * P:(q + 1) * QK * P, :]
                    .rearrange("(k p) i -> p k i", p=P))
            sdw = consts.tile([P, ISM, h], BF, name="sdw")
            nc.scalar.dma_start(sdw[:],
                                swd.rearrange("(k p) i -> p k i", p=P))

            wtk_sb = consts.tile([P, sum(CT)], F32, name="wtk_sb")
            nc.scalar.dma_start(wtk_sb[:], wtk[:, :])

            gss = consts.tile([P, ISM, tt], BF, name="gss")
            ass = consts.tile([P, ISM, tt], BF, name="ass")

            TN = TH2 // 512        # 512-token chunks per half
            for th in range(2):
                t0 = th * TH2
                psg = [[psum.tile([P, 512], F32, name="ps", tag="ps")
                        for _ in range(TN)] for _ in range(ISM)]
                psu = [[psum.tile([P, 512], F32, name="ps", tag="ps")
                        for _ in range(TN)] for _ in range(ISM)]
                for k0 in range(KH // QK):
                    hsl = htp.tile([P, QK, TH2], BF, name="hsl",
                                   tag="hsl")
                    nc.sync.dma_start(
                        hsl, ht[k0 * QK * P:(k0 + 1) * QK * P,
                                t0:t0 + TH2]
                        .rearrange("(q p) t -> p q t", p=P))
                    for kq in range(QK):
                        k = k0 * QK + kq
                        for m in range(ISM):
                            for tn in range(TN):
                                nc.tensor.matmul(
                                    psg[m][tn],
                                    sgw[:, k, m * P:(m + 1) * P],
                                    hsl[:, kq, tn * 512:(tn + 1) * 512],
                                    start=(k == 0), stop=(k == KH - 1))
                                nc.tensor.matmul(
                                    psu[m][tn],
                                    suw[:, k, m * P:(m + 1) * P],
                                    hsl[:, kq, tn * 512:(tn + 1) * 512],
                                    start=(k == 0), stop=(k == KH - 1))
                for m in range(ISM):
                    for tn in range(TN):
                        sl_ = slice(t0 + tn * 512, t0 + (tn + 1) * 512)
                        nc.scalar.activation(gss[:, m, sl_], psg[m][tn][:],
                                             ACT.Silu)
                        nc.vector.tensor_mul(ass[:, m, sl_], gss[:, m, sl_],
                                             psu[m][tn][:])

            # shared down-projection: token-major [tt, h] partial
            # (DVE eviction + sync-queue output keep ACT free for the
            # routed phase's silu/scale work)
            for cm in range(tt // P):
                for hn in range(HN):
                    pst = psum.tile([P, 512], F32, name="ps", tag="ps")
                    for kk in range(ISM):
                        nc.tensor.matmul(
                            pst, ass[:, kk, cm * P:(cm + 1) * P],
                            sdw[:, kk, hn * 512:(hn + 1) * 512],
                            start=(kk == 0), stop=(kk == ISM - 1))
                    st = stage.tile([P, 512], F32, name="st", tag="st")
                    nc.vector.tensor_copy(st[:], pst[:])
                    nc.sync.dma_start(
                        ysh[cm * P:(cm + 1) * P, hn * 512:(hn + 1) * 512],
                        st[:])

            # ---------------- routed experts (expert-parallel) ------------
            for e in range(el):
                C = caps[e]
                nch = _chunks(C, 512)
                # gate projection
                wgt = wpool.tile([P, KH, im], BF, name="wmat", tag="wmat")
                nc.sync.dma_start(wgt[:],
                                  wg[e].rearrange("(k p) i -> p k i", p=P))
                gst = gpool.tile([P, MI, caps[0]], BF, name="gst")
                for (noff, nsz) in nch:
                    pse = [psum.tile([P, 512], F32, name="ps", tag="ps")
                           for _ in range(MI)]
                    for k0 in range(KH // QK):
                        xsl = xpool.tile([P, QK, 512], BF, name="xsl",
                                         tag="xsl")[:, :, :nsz]
                        nc.sync.dma_start(
                            xsl, xt[k0 * QK * P:(k0 + 1) * QK * P,
                                    coff[e] + noff:coff[e] + noff + nsz]
                            .rearrange("(q p) n -> p q n", p=P))
                        for kq in range(QK):
                            k = k0 * QK + kq
                            for m in range(MI):
                                nc.tensor.matmul(
                                    pse[m][:, :nsz],
                                    wgt[:, k, m * P:(m + 1) * P],
                                    xsl[:, kq, :],
                                    start=(k == 0), stop=(k == KH - 1))
                    for m in range(MI):
                        nc.scalar.activation(gst[:, m, noff:noff + nsz],
                                             pse[m][:, :nsz], ACT.Silu)

                # up projection (evicts through silu(G) * U)
                wut = wpool.tile([P, KH, im], BF, name="wmat", tag="wmat")
                nc.sync.dma_start(wut[:],
                                  wu[e].rearrange("(k p) i -> p k i", p=P))
                ast = apool.tile([P, MI, caps[0]], BF, name="ast")
                for (noff, nsz) in nch:
                    pse = [psum.tile([P, 512], F32, name="ps", tag="ps")
                           for _ in range(MI)]
                    for k0 in range(KH // QK):
                        xsl = xpool.tile([P, QK, 512], BF, name="xsl",
                                         tag="xsl")[:, :, :nsz]
                        nc.scalar.dma_start(
                            xsl, xt[k0 * QK * P:(k0 + 1) * QK * P,
                                    coff[e] + noff:coff[e] + noff + nsz]
                            .rearrange("(q p) n -> p q n", p=P))
                        for kq in range(QK):
                            k = k0 * QK + kq
                            for m in range(MI):
                                nc.tensor.matmul(
                                    pse[m][:, :nsz],
                                    wut[:, k, m * P:(m + 1) * P],
                                    xsl[:, kq, :],
                                    start=(k == 0), stop=(k == KH - 1))
                    for m in range(MI):
                        nc.vector.tensor_mul(ast[:, m, noff:noff + nsz],
                                             gst[:, m, noff:noff + nsz],
                                             pse[m][:, :nsz])

                # down projection, token-major out, combine-weight scaling
                wdt = wpool.tile([P, MI, h], BF, name="wmat", tag="wmat")
                nc.sync.dma_start(wdt[:],
                                  wd[e].rearrange("(k p) i -> p k i", p=P))
                for cm in range(C // P):
                    for hn in range(HN):
                        pst = psum.tile([P, 512], F32, name="ps", tag="ps")
                        for kk in range(MI):
                            nc.tensor.matmul(
                                pst, ast[:, kk, cm * P:(cm + 1) * P],
                                wdt[:, kk, hn * 512:(hn + 1) * 512],
                                start=(kk == 0), stop=(kk == MI - 1))
                        st = stage.tile([P, 512], F32, name="st", tag="st")
                        nc.scalar.activation(
                            st[:], pst[:], ACT.Copy,
                            scale=wtk_sb[:, ctoff[e] + cm:ctoff[e] + cm + 1])
                        nc.sync.dma_start(
                            yex[e][cm * P:(cm + 1) * P,
                                   hn * 512:(hn + 1) * 512], st[:])

    nc.compile()
    return nc


_prog_cache = {}

# Debug/timing hooks for the local test harness (harmless defaults for
# grading: no tracing, results kept only when asked for).
TRACE = False
TRACE_KWARGS = {}
LAST_RESULT = None


def _get_program(caps):
    if caps not in _prog_cache:
        _prog_cache[caps] = build_program(caps)
    return _prog_cache[caps]


def _route(h32, gate_w):
    """Host router: returns per-expert (token_idx, combine_weight)."""
    logits = h32 @ np.asarray(gate_w, np.float32).T            # [T, E]
    rw = 1.0 / (1.0 + np.exp(-logits))
    topk_idx = np.argsort(-rw, axis=-1, kind="stable")[:, :TOPK]
    topk_w = np.take_along_axis(rw, topk_idx, -1)
    topk_w = topk_w / (topk_w.sum(-1, keepdims=True) + 1e-6) * SCALE
    sel, wsel = [], []
    for e in range(E):
        mask = topk_idx == e                                    # [T, K]
        tok = np.nonzero(mask.any(-1))[0]
        sel.append(tok)
        wsel.append((topk_w * mask).sum(-1)[tok].astype(np.float32))
    return sel, wsel


def kernel(hidden_states, gate_w, Wg, Wu, Wd, sWg, sWu, sWd):
    h32 = np.asarray(hidden_states, np.float32).reshape(T, H)
    sel, wsel = _route(h32, gate_w)

    # Assign experts to (core, slot): slot 0 gets the 8 busiest experts so
    # slot capacities (compile-time constants) hug the actual counts.
    order = sorted(range(E), key=lambda e: -len(sel[e]))
    slot_experts = [order[:NCORES], order[NCORES:]]             # [slot][core]
    caps = tuple(
        max(P, int(math.ceil(max(len(sel[e]) for e in slot_experts[s]) / P))
            * P)
        for s in range(EL))
    nc = _get_program(caps)
    coff = [sum(caps[:i]) for i in range(EL)]
    CT = [c // P for c in caps]
    ctoff = [sum(CT[:i]) for i in range(EL)]

    ht_bf = np.ascontiguousarray(h32.T).astype(BF16)            # [H, T]
    swg_bf = np.asarray(sWg, np.float32).astype(BF16)
    swu_bf = np.asarray(sWu, np.float32).astype(BF16)
    swd_bf = np.asarray(sWd, np.float32).astype(BF16)
    wg_bf = np.asarray(Wg, np.float32).astype(BF16)
    wu_bf = np.asarray(Wu, np.float32).astype(BF16)
    wd_bf = np.asarray(Wd, np.float32).astype(BF16)
    ISS = IS // 4
    TT = T // 2

    in_maps = []
    for c in range(NCORES):
        experts = [slot_experts[s][c] for s in range(EL)]
        si, tb = c // 2, c % 2          # shared expert: IS slice, token half
        xt = np.zeros((H, sum(caps)), BF16)
        wtk = np.zeros((P, sum(CT)), np.float32)
        for s, e in enumerate(experts):
            tok, w = sel[e], wsel[e]
            n = len(tok)
            xt[:, coff[s]:coff[s] + n] = ht_bf[:, tok]
            wcol = np.zeros(caps[s], np.float32)
            wcol[:n] = w
            wtk[:, ctoff[s]:ctoff[s] + CT[s]] = wcol.reshape(CT[s], P).T
        in_maps.append({
            "xt": xt,
            "wtk": wtk,
            "ht": np.ascontiguousarray(ht_bf[:, tb * TT:(tb + 1) * TT]),
            "wg": np.ascontiguousarray(wg_bf[experts]),
            "wu": np.ascontiguousarray(wu_bf[experts]),
            "wd": np.ascontiguousarray(wd_bf[experts]),
            "swg": np.ascontiguousarray(swg_bf[:, si * ISS:(si + 1) * ISS]),
            "swu": np.ascontiguousarray(swu_bf[:, si * ISS:(si + 1) * ISS]),
            "swd": np.ascontiguousarray(swd_bf[si * ISS:(si + 1) * ISS, :]),
        })

    res = run_bass_kernel_spmd(nc, in_maps, list(range(NCORES)),
                               trace=TRACE, **TRACE_KWARGS)
    if TRACE:
        global LAST_RESULT
        LAST_RESULT = res

    out = np.empty((T, H), np.float32)
    for tb in range(2):
        acc = res.results[tb]["ysh"].astype(np.float32)
        for si in range(1, 4):
            acc += res.results[si * 2 + tb]["ysh"]
        out[tb * TT:(tb + 1) * TT] = acc
    for c in range(NCORES):
        for s in range(EL):
            e = slot_experts[s][c]
            tok = sel[e]
            out[tok] += res.results[c][f"yex{s}"][:len(tok)]

    return out.reshape(np.asarray(hidden_states).shape).astype(np.float32)
